# revision 1
# baseline (speedup 1.0000x reference)
"""Trainium2 Bass kernel for nn_DctAtt (B=32, D=1024, N=4096, K=5).

The reference collapses to att[b,d] = x[b,d,:] . w  (w = C @ dw_w precomputed
on host), followed by tiny [32,1024] BN/GELU/softmax work done on host.
The device kernel streams x (512 MiB, data-parallel over B across 8 cores,
64 MiB/core) through fused DVE AFFINE_MUL_REDUCE dot products.

Trace findings this design is built on (ntff profiles; see also
memory/trn2-dma-engine-findings.md):
  * A [128, 4096] f32 tile from a contiguous 2 MiB DRAM block is moved as
    128 contiguous 16 KiB descriptors, distributed ceil(L/16) lines per
    SDMA engine from engine 0; each engine sustains ~26.5 GB/s (aggregate
    ~425 GB/s, the practical per-core ceiling).
  * ONLY full-128-partition, source-contiguous transfers hit that rate.
    Partial-partition dma_starts (L < 128) take a ~2x slower descriptor
    path (~13 GB/s/engine) on sync, scalar AND gpsimd queues, serialized
    or pipelined; column-sliced (strided-source) transfers drop to
    ~17-20 GB/s. This rules out de-weighting individual engines by
    skipping their partitions -- every such variant measured slower.
  * SDMA engine 15 intermittently (and in most executions on this box)
    runs at ~22 GB/s instead of 26.5. Uniform striping means the stream
    drains at the slowest engine's pace: exec is bimodal ~176/~213 us.
    No structural mitigation survives the partial-transfer penalty above.
  * The old [128, 4096] w replica cost a 2 MiB HBM read serialized ahead
    of the x stream (~6 us). Now w is loaded as one 16 KiB row and
    broadcast across partitions on-chip with a K=1 TensorE matmul
    (ones[1,128]^T @ w[1,512] per PSUM bank), then ACT-copied to SBUF --
    zero SDMA work, bit-exact in f32.
  * DVE fp32 tensor-tensor ops are capped at 1 elem/cycle/lane (4.42 us
    per [128, 4096] reduction); DVE total ~141 us < the ~158 us stream,
    so only the last tile's 4.4 us trails the final DMA byte.
  * ~7 us of framework preamble and ~9 us of teardown (257 NEFF-level
    per-semaphore clears + barriers) sit inside the measured exec window
    regardless of kernel structure.

Unit plan (per core, 4096 rows): 32 uniform tiles of [128, 4096], rows
assigned sequentially so every dma_start reads a contiguous 2 MiB DRAM
block. y_sb[p, t] = dot(row 128t+p, w).
"""

import math

import numpy as np

import concourse.bacc as bacc
import concourse.mybir as mybir
import concourse.tile as tile
from concourse import bass_utils

# Problem constants (hardcoded: the grading harness ships only this file).
B, D, N = 32, 1024, 4096
K = 5
BN_EPS = 1e-5
N_CORES = 8
P = 128
ROWS_PER_CORE = (B // N_CORES) * D  # 4096

import os as _os

# In-flight [128, 4096] tiles (16 KiB/partition each).
XP_BUFS = int(_os.environ.get("DCT_BUFS", "8"))
# 1: de-weight SDMA engine 15 via slim tiles; 0: uniform 32 full tiles.
# NOTE: probe2 showed partial-partition dma_starts take a ~2x slower
# descriptor path (13 vs 26.9 GB/s per engine), so slim tiles as
# partial-partition transfers are a net loss; default off.
SLIM = int(_os.environ.get("DCT_SLIM", "0"))
# 2: broadcast w via PE and leave it resident in PSUM (crashes at runtime:
#    DVE fp32 reads spanning PSUM banks are not supported); 1: PE broadcast
#    + ACT copy to SBUF (default); 0: read a [128, N] replica from HBM.
PEW = int(_os.environ.get("DCT_PEW", "1"))
# 1: native TENSOR_TENSOR_REDUCE (crashes at runtime in this stack, and fp32
#    tensor-tensor is 1 elem/cycle/lane either way); 0: AFFINE_MUL_REDUCE.
TTR = int(_os.environ.get("DCT_OP", "0"))
# Partitions served by the slow SDMA engine (from probe.py); slim tiles
# skip exactly these.
EXCL = tuple(
    int(p) for p in _os.environ.get("DCT_EXCL", "92,93,94,95,124,125,126,127").split(",")
)
# Wide tiles [128, 8192] (2 consecutive rows per partition, 32 KiB
# contiguous lines) move ~0.8% faster per engine (26.8 vs 26.57 GB/s) but
# make DVE consumption coarser: DVE runs 8.84 us per wide tile against a
# 9.65 us arrival and ends ~8.8 us behind the last byte.
# 1: 15 wide + 2 narrow tiles (measured +2 us: DVE can't drain).
# 2: 10 wide then 12 narrow: DVE drains ~0.41 us per narrow tile, so the
#    tail is back to one narrow reduction while the wide bulk keeps the
#    faster line rate.
WIDE = int(_os.environ.get("DCT_WIDE", "0"))
# 1: alternate x tiles between the sync and scalar HWDGE rings (two
# descriptor streams per SDMA engine, trigger issue split across SP/ACT).
DUALQ = int(_os.environ.get("DCT_DUALQ", "0"))
# 1: cast x f32->bf16 inline in the SDMA datapath (SWDGE/gpsimd queue;
# HBM read bytes unchanged) and reduce in bf16 with f32 accumulation.
# Halves SBUF write traffic and the DVE tail if bf16 runs 2x/cycle.
BF16 = int(_os.environ.get("DCT_BF16", "0"))
# 1: compile with target_bir_lowering=True (different NEFF packaging; the
# ~6 us of per-semaphore teardown clears is injected at that layer).
LOWER = int(_os.environ.get("DCT_LOWER", "0"))


def _spans_excluding(excl):
    """Partition spans [p0, p1) covering 0..127 minus `excl`."""
    excl = set(excl)
    spans = []
    p = 0
    while p < P:
        if p in excl:
            p += 1
            continue
        q = p
        while q < P and q not in excl:
            q += 1
        spans.append((p, q - p))
        p = q
    return tuple(spans)


FULL_SPANS = ((0, P),)


def _unit_plan():
    """Static tile list shared by the device build and the host gather.

    tiles[i] = dict(row0, spans, wide, ycol); rows are assigned
    sequentially, so each span's DMA reads a contiguous DRAM block.
    Wide tiles hold 256 rows (partition p <- rows 2p, 2p+1; 32 KiB
    contiguous lines) and use two y columns.
    """
    if WIDE:
        n_wide = 15 if WIDE == 1 else 10
        tiles = [
            {"row0": 256 * t, "spans": FULL_SPANS, "wide": True, "ycol": 2 * t}
            for t in range(n_wide)
        ] + [
            {"row0": 256 * n_wide + 128 * i, "spans": FULL_SPANS,
             "wide": False, "ycol": 2 * n_wide + i}
            for i in range(32 - 2 * n_wide)
        ]
    elif SLIM:
        slim_spans = _spans_excluding(EXCL)
        tiles = []
        cur = 0
        for t in range(33):
            spans = slim_spans if (t % 2 == 1 and t < 32) else FULL_SPANS
            cap = sum(c for _, c in spans)
            tiles.append({"row0": cur, "spans": spans, "wide": False, "ycol": t})
            cur += cap
        assert cur == ROWS_PER_CORE, cur
    else:
        tiles = [
            {"row0": 128 * t, "spans": FULL_SPANS, "wide": False, "ycol": t}
            for t in range(32)
        ]
    return tiles


_compiled_nc = None


def _build():
    """Build + compile the per-core Bass program (cached per process)."""
    global _compiled_nc
    if _compiled_nc is not None:
        return _compiled_nc

    tiles = _unit_plan()
    n_ycols = max(u["ycol"] + (2 if u["wide"] else 1) for u in tiles)
    nc = bacc.Bacc(
        "TRN2",
        target_bir_lowering=bool(LOWER),
        debug=False,
        enable_asserts=False,
        num_devices=N_CORES,
    )
    f32 = mybir.dt.float32
    assert not BF16 or (PEW == 1 and not WIDE), "BF16 needs PEW=1, narrow tiles"
    xd = mybir.dt.bfloat16 if BF16 else f32  # on-chip x / w operand dtype
    x_sh = nc.dram_tensor("x_sh", [ROWS_PER_CORE, N], f32, kind="ExternalInput").ap()
    if PEW:
        w_in = nc.dram_tensor("w_row", [1, N], f32, kind="ExternalInput").ap()
    else:
        w_in = nc.dram_tensor("w_rep", [P, N], f32, kind="ExternalInput").ap()
    y_out = nc.dram_tensor("y_out", [P, n_ycols], f32, kind="ExternalOutput").ap()

    xp_bufs = min(XP_BUFS, 5) if WIDE else XP_BUFS  # wide tiles: 32 KiB/part
    with tile.TileContext(nc) as tc:
        with (
            tc.tile_pool(name="wp", bufs=1) as wp,
            tc.tile_pool(name="xp", bufs=xp_bufs) as xp,
            tc.tile_pool(name="pw", bufs=1, space="PSUM") as pwp,
        ):
            cw = 512  # one PSUM bank of f32
            if PEW:
                # w: one 16 KiB HBM read on the Scalar HWDGE ring, then
                # partition-broadcast through the PE (ones[1,128].T @ w[1,c])
                # into PSUM. Zero SDMA cost vs the 2 MiB replica read that
                # used to head the x stream. PEW=2 leaves w resident in
                # PSUM (all 8 banks) and the DVE reads in1 from there.
                w_row = wp.tile([1, N], f32)
                ones = wp.tile([1, P], f32)
                nc.scalar.dma_start(out=w_row, in_=w_in)
                nc.vector.memset(ones, 1.0)
                w_ps = pwp.tile([P, N], f32)
                for c in range(N // cw):
                    nc.tensor.matmul(
                        w_ps[:, c * cw : (c + 1) * cw],
                        ones,
                        w_row[:, c * cw : (c + 1) * cw],
                        start=True,
                        stop=True,
                    )
                if PEW == 2:
                    w_op = w_ps
                else:
                    # ACT copy PSUM -> SBUF; casts to bf16 when BF16.
                    w_sb = wp.tile([P, N], xd)
                    for c in range(N // cw):
                        nc.scalar.copy(
                            out=w_sb[:, c * cw : (c + 1) * cw],
                            in_=w_ps[:, c * cw : (c + 1) * cw],
                        )
                    w_op = w_sb
            else:
                w_sb = wp.tile([P, N], f32)
                nc.scalar.dma_start(out=w_sb, in_=w_in)
                w_op = w_sb
            y_sb = wp.tile([P, n_ycols], f32)
            # Stride-0 free dim: the fused op's elementwise product is not
            # materialised (every element lands on the same column).
            dummy = wp.tile([P, 1], xd)
            def reduce_into(xin, ycol):
                # accum = sum(x * w) per partition = row dot. Slim tiles
                # leave the excluded partitions' stale contents in place;
                # they only pollute y rows the host never reads.
                if TTR:
                    nc.vector.tensor_tensor_reduce(
                        out=dummy.broadcast_to((P, N)),
                        in0=xin,
                        in1=w_op,
                        scale=1.0,
                        scalar=0.0,
                        op0=mybir.AluOpType.mult,
                        op1=mybir.AluOpType.add,
                        accum_out=y_sb[:, ycol : ycol + 1],
                    )
                else:
                    nc.vector.affine_mul_reduce(
                        out=dummy.broadcast_to((P, N)),
                        accum_out=y_sb[:, ycol : ycol + 1],
                        in0=xin,
                        in1=w_op,
                        scale=1.0,
                        bias=0.0,
                    )

            for ti, ut in enumerate(tiles):
                row0, spans, ycol = ut["row0"], ut["spans"], ut["ycol"]
                if BF16:
                    eng = nc.gpsimd  # dtype cast during DMA is SWDGE-only
                else:
                    eng = nc.scalar if (DUALQ and ti % 2 == 1) else nc.sync
                if ut["wide"]:
                    xt = xp.tile([P, 2 * N], f32)
                    eng.dma_start(
                        out=xt,
                        in_=x_sh[row0 : row0 + 2 * P, :].rearrange(
                            "(p h) n -> p (h n)", h=2
                        ),
                    )
                    reduce_into(xt[:, 0:N], ycol)
                    reduce_into(xt[:, N : 2 * N], ycol + 1)
                else:
                    xt = xp.tile([P, N], xd)
                    r = row0
                    for p0, cnt in spans:
                        eng.dma_start(
                            out=xt[p0 : p0 + cnt, :], in_=x_sh[r : r + cnt, :]
                        )
                        r += cnt
                    reduce_into(xt, ycol)
            # Single y store: a split store (bulk columns early, tail late)
            # measured ~1 us slower -- the mid-stream trigger interferes
            # with the x-tile queue and the tail store pays full latency.
            nc.sync.dma_start(out=y_out, in_=y_sb)

    nc.compile()
    _compiled_nc = nc
    return nc


def _dct_weight(dw_w):
    """w = C @ dw_w in float64, where C is the [N, K] ortho DCT-II basis."""
    n = np.arange(N, dtype=np.float64)
    k = np.arange(K, dtype=np.float64)
    C = np.cos(np.pi * (2.0 * n[:, None] + 1.0) * k[None, :] / (2.0 * N))
    C *= math.sqrt(2.0 / N)
    C[:, 0] *= 1.0 / math.sqrt(2.0)
    return (C @ np.asarray(dw_w, dtype=np.float64)).astype(np.float32)


def _erf(x):
    try:
        from scipy.special import erf

        return erf(x)
    except Exception:
        return np.vectorize(math.erf)(x).astype(x.dtype)


def _gather_att_core(y):
    """y_out [P, n_ycols] -> per-core att rows [ROWS_PER_CORE]."""
    tiles = _unit_plan()
    att = np.empty(ROWS_PER_CORE, dtype=np.float32)
    for ut in tiles:
        r, yc = ut["row0"], ut["ycol"]
        if ut["wide"]:
            att[r : r + 2 * P : 2] = y[:, yc]
            att[r + 1 : r + 2 * P : 2] = y[:, yc + 1]
        else:
            for p0, cnt in ut["spans"]:
                att[r : r + cnt] = y[p0 : p0 + cnt, yc]
                r += cnt
    return att


def _run_device(inputs, trace=False, **spmd_kwargs):
    """Run the dot-product phase on the 8 cores; return att [B, D] (pre-BN)
    and the BassKernelResults (for profiling from test harnesses)."""
    x = np.ascontiguousarray(np.asarray(inputs["x"], dtype=np.float32))
    w = _dct_weight(inputs["dw_w"])
    if PEW:
        w_name, w_val = "w_row", np.ascontiguousarray(w.reshape(1, N))
    else:
        w_name, w_val = "w_rep", np.ascontiguousarray(
            np.broadcast_to(w[None, :], (P, N))
        )

    nc = _build()
    b_per_core = B // N_CORES
    in_maps = []
    for c in range(N_CORES):
        xs = np.ascontiguousarray(
            x[c * b_per_core : (c + 1) * b_per_core].reshape(ROWS_PER_CORE, N)
        )
        in_maps.append({"x_sh": xs, w_name: w_val})

    res = bass_utils.run_bass_kernel_spmd(
        nc, in_maps, core_ids=list(range(N_CORES)), trace=trace, **spmd_kwargs
    )
    att = np.concatenate(
        [_gather_att_core(res.results[c]["y_out"]) for c in range(N_CORES)]
    ).reshape(B, D)
    return att, res


def _postprocess(att, inputs):
    """Host tail on the tiny [B, D] array: +dw_b, BatchNorm (global batch
    stats, training mode), exact GELU, 1x1 conv affine, softmax over D."""
    dw_b = np.float32(np.asarray(inputs["dw_b"]).reshape(-1)[0])
    gamma = np.float32(np.asarray(inputs["gamma"]).reshape(-1)[0])
    beta = np.float32(np.asarray(inputs["beta"]).reshape(-1)[0])
    conv_w = np.float32(np.asarray(inputs["conv_w"]).reshape(-1)[0])
    conv_b = np.float32(np.asarray(inputs["conv_b"]).reshape(-1)[0])

    att = att.astype(np.float32) + dw_b
    mean = att.mean(dtype=np.float64)
    var = np.mean((att.astype(np.float64) - mean) ** 2)
    inv_std = np.float32(1.0 / math.sqrt(var + BN_EPS))
    att = (att - np.float32(mean)) * inv_std * gamma + beta
    # Exact GELU: x * 0.5 * (1 + erf(x / sqrt(2)))
    att = (att * 0.5 * (1.0 + _erf(att / np.float32(math.sqrt(2.0))))).astype(
        np.float32
    )
    att1 = att * conv_w + conv_b
    att1 = att1 - att1.max(axis=-1, keepdims=True)
    e = np.exp(att1.astype(np.float32))
    att1 = (e / e.sum(axis=-1, keepdims=True)).astype(np.float32)
    att1 = att1[:, :, None]
    return att1, (np.float32(1.0) - att1).astype(np.float32)


def kernel(**inputs):
    att, _ = _run_device(inputs)
    return _postprocess(att, inputs)

